# revision 25
# baseline (speedup 1.0000x reference)
"""Trainium2 Bass kernel for EuclideanDistLoss.

reference:
    diff = latent1 - latent2                  # [B, D]
    d = sqrt(sum(diff^2, axis=1))             # [B]
    dev = d - CUTOFF
    penalty = where(dev > 0, dev^2, PRESSURE * dev^2)
    return mean(penalty)

Strategy: data-parallel over the batch dim across 8 NeuronCores. Each core
streams its 32768x256 shard of both inputs through SBUF ([128, k*256] tiles,
k rows per partition), computes per-sample sum-of-squares via DVE subtract ->
ACT Square -> DVE grouped 3D reduce into ssq[128, 256], then one penalty
chain (ACT Sqrt -> DVE cmp/blend -> ACT Square -> DVE mult+reduce) produces
a per-partition partial sum, which the idle GpSimd engine all-reduces
across partitions so a single 4-byte element is DMA'd out per core. The
host sums the 8 scalars in float64 and divides by the global batch (the
"all-reduce" of the scalar).

DMA: the two input streams go through BOTH HWDGE queue sets (a via the SP
engine's qSpDynamicHW, b via the Activation engine's qActDynamicHW), uniform
k=8 tiles (8KB per partition per transfer). This tracks the concurrent
DMA-only bandwidth ceiling (~188-194 us/pass/core, ~355 GB/s/core vs 360
roofline) in every session measured, whereas routing both streams through
one queue set drifts between 190 and 232 us across sessions.

The benchmark (repeat>1) kernels run the identical per-pass instruction
stream, with the penalty chain software-pipelined one pass behind on a
double-buffered ssq: pass r's chain is emitted a few tiles into pass r+1,
and its single-descriptor out-DMA one pass later still, so every
instruction's inputs are long since computed and nothing ever stalls the
in-order HWDGE load queues (measured: chain + out-DMA per pass is free —
the kernel times identically to a DMA-only probe).
"""

import numpy as np

B, D = 262144, 256
N_CORES = 8
P = 128
CUTOFF = 0.1
PRESSURE = 10.0

B_LOCAL = B // N_CORES  # 32768
K_DEFAULT = 8           # rows per partition per tile -> 8KB contiguous / partition
BUFS_DEFAULT = 8
ENGINES_DEFAULT = ("sync", "act")
EMIT_AT = 4             # tile index of pass r+1 where pass r's chain is emitted


def build_nc(b_local=B_LOCAL, k=K_DEFAULT, repeat=1, bufs=BUFS_DEFAULT,
             compute=True, engines=ENGINES_DEFAULT, per_pass_chain=True,
             chain_out_dma=True):
    """Build + compile the per-core Bass program (SPMD: same program on all cores).

    repeat>1 re-runs the whole pass over the same data (for benchmarking:
    slope of time vs repeat isolates pure on-device time; each pass includes
    the full penalty chain via software pipelining).
    compute=False builds a DMA-only variant (bandwidth ceiling probe).
    """
    import concourse.bacc as bacc
    import concourse.tile as tile
    from concourse import mybir
    from concourse import bass_isa

    f32 = mybir.dt.float32
    Alu = mybir.AluOpType
    Act = mybir.ActivationFunctionType

    if isinstance(k, int):
        tile_rows = P * k
        assert b_local % tile_rows == 0
        schedule = [k] * (b_local // tile_rows)
    else:  # explicit per-tile k schedule
        schedule = list(k)
        assert sum(schedule) * P == b_local
    n = sum(schedule)  # k-units per partition (= penalties per partition)

    nc = bacc.Bacc("TRN2", target_bir_lowering=False, debug=False,
                   num_devices=N_CORES)
    # DMA issuing engines for the a/b input streams. "sync" = SP HWDGE queue
    # set (qSpDynamicHW), "act" = Activation HWDGE queue set (qActDynamicHW),
    # "gpsimd" = Pool SWDGE.
    def _eng(name):
        return {"sync": nc.sync, "act": nc.scalar, "gpsimd": nc.gpsimd}[name]

    a = nc.dram_tensor("latent1", [b_local, D], f32, kind="ExternalInput").ap()
    b = nc.dram_tensor("latent2", [b_local, D], f32, kind="ExternalInput").ap()
    out = nc.dram_tensor("out", [1, 1], f32, kind="ExternalOutput").ap()

    with tile.TileContext(nc) as tc:
        with (
            tc.tile_pool(name="pa", bufs=bufs) as pa,
            tc.tile_pool(name="pb", bufs=bufs) as pb,
            tc.tile_pool(name="keep", bufs=1) as keep,
        ):
            # ssq double-buffered so pass r+1 can stream into one buffer
            # while pass r's penalty chain reads the other.
            ssqs = [keep.tile([P, n], f32, name=f"ssq{i}") for i in range(2)]
            d_ = keep.tile([P, n], f32)
            mask = keep.tile([P, n], f32)  # 1.0 where d < CUTOFF
            fac = keep.tile([P, n], f32)   # 1 + (PRESSURE-1)*mask
            dd = keep.tile([P, n], f32)    # (d - CUTOFF)^2
            pen = keep.tile([P, n], f32)
            # psum/red double-buffered: chain(r) writes psums[r%2], then the
            # idle GpSimd engine all-reduces across partitions into
            # reds[r%2] (every partition holds the core's total). The out-DMA
            # moves ONE 4-byte element, one pass later on the SP queue, when
            # the data is guaranteed ready: a [128,1] out-DMA fragments into
            # 128 tiny descriptors and measured +15us/pass regardless of
            # queue or timing; the single-descriptor form is free.
            psums = [keep.tile([P, 1], f32, name=f"psum{i}") for i in range(2)]
            reds = [keep.tile([P, 1], f32, name=f"red{i}") for i in range(2)]
            neg_cut = keep.tile([P, 1], f32)
            nc.vector.memset(neg_cut, -CUTOFF)

            def penalty_chain(buf):
                # critical path: Sqrt -> Square (both ACT, one table set) ->
                # mult -> reduce; mask/fac run on DVE in parallel with Square.
                ssq = ssqs[buf]
                nc.scalar.activation(out=d_, in_=ssq, func=Act.Sqrt)
                nc.vector.tensor_scalar(mask, d_, CUTOFF, None, Alu.is_lt)
                nc.vector.tensor_scalar(
                    fac, mask, PRESSURE - 1.0, 1.0, Alu.mult, Alu.add
                )
                nc.scalar.activation(
                    out=dd, in_=d_, func=Act.Square, bias=neg_cut[:]
                )
                nc.vector.tensor_tensor(out=pen, in0=dd, in1=fac, op=Alu.mult)
                nc.vector.tensor_reduce(
                    out=psums[buf], in_=pen, axis=mybir.AxisListType.X,
                    op=Alu.add,
                )
                nc.gpsimd.partition_all_reduce(
                    reds[buf][:], psums[buf][:], 128, bass_isa.ReduceOp.add
                )

            if not compute:
                nc.vector.memset(reds[0], 0.0)
                nc.sync.dma_start(out=out, in_=reds[0][0:1, :])
            for r in range(repeat):
                cur = r % 2
                r0 = 0   # row offset within the shard
                c0 = 0   # column offset within ssq
                for i, kt in enumerate(schedule):
                    # partition p holds kt consecutive rows -> contiguous
                    # kt*1KB per partition
                    a_v = a[r0:r0 + P * kt, :].rearrange("(p k) d -> p (k d)", p=P)
                    b_v = b[r0:r0 + P * kt, :].rearrange("(p k) d -> p (k d)", p=P)
                    ta = pa.tile([P, kt * D], f32, tag="ta")
                    tb = pb.tile([P, kt * D], f32, tag="tb")
                    if engines == "split":
                        # each stream split across both queue sets by
                        # partition halves (both queues stay load-balanced
                        # even if one set runs slow)
                        h = P // 2
                        nc.sync.dma_start(out=ta[0:h, :], in_=a_v[0:h, :])
                        nc.scalar.dma_start(out=ta[h:P, :], in_=a_v[h:P, :])
                        nc.scalar.dma_start(out=tb[0:h, :], in_=b_v[0:h, :])
                        nc.sync.dma_start(out=tb[h:P, :], in_=b_v[h:P, :])
                    else:
                        pair = (engines if isinstance(engines, tuple)
                                else engines[i % len(engines)])
                        _eng(pair[0]).dma_start(out=ta, in_=a_v)
                        _eng(pair[1]).dma_start(out=tb, in_=b_v)
                    r0 += P * kt
                    if not compute:
                        continue
                    nc.vector.tensor_tensor(out=ta, in0=ta, in1=tb, op=Alu.subtract)
                    nc.scalar.activation(out=ta, in_=ta, func=Act.Square)
                    nc.vector.tensor_reduce(
                        out=ssqs[cur][:, c0:c0 + kt],
                        in_=ta.rearrange("p (k d) -> p k d", d=D),
                        axis=mybir.AxisListType.X,
                        op=Alu.add,
                    )
                    c0 += kt
                    if i == EMIT_AT and per_pass_chain and compute:
                        if r >= 2 and chain_out_dma:
                            # chain(r-2)'s total: computed a full pass ago,
                            # single-descriptor DMA, ready on arrival
                            nc.sync.dma_start(out=out, in_=reds[r % 2][0:1, :])
                        if r >= 1:
                            # previous pass's penalty chain: all inputs
                            # ready, hidden under this pass's stream
                            penalty_chain(1 - cur)
            if compute:
                penalty_chain((repeat - 1) % 2)
                nc.sync.dma_start(out=out, in_=reds[(repeat - 1) % 2][0:1, :])

    nc.compile()
    return nc


_NC_CACHE = {}


def _get_nc():
    key = "default"
    if key not in _NC_CACHE:
        _NC_CACHE[key] = build_nc()
    return _NC_CACHE[key]


def run_spmd(latent1, latent2, trace=False, **kwargs):
    """Shard inputs, run on 8 cores, return (scalar_loss, BassKernelResults)."""
    from concourse.bass_utils import run_bass_kernel_spmd

    nc = _get_nc()
    a = np.ascontiguousarray(np.asarray(latent1, dtype=np.float32))
    b = np.ascontiguousarray(np.asarray(latent2, dtype=np.float32))
    assert a.shape == (B, D) and b.shape == (B, D)
    in_maps = [
        {
            "latent1": a[c * B_LOCAL:(c + 1) * B_LOCAL],
            "latent2": b[c * B_LOCAL:(c + 1) * B_LOCAL],
        }
        for c in range(N_CORES)
    ]
    res = run_bass_kernel_spmd(
        nc, in_maps, core_ids=list(range(N_CORES)), trace=trace, **kwargs
    )
    # each core returns its shard's penalty total in out[0, 0]; the host
    # all-reduces the 8 scalars and divides by the global batch
    total = sum(np.asarray(r["out"], dtype=np.float64).sum() for r in res.results)
    return np.asarray(total / B, dtype=np.float32), res


def kernel(latent1, latent2):
    loss, _ = run_spmd(latent1, latent2)
    return loss


# revision 29
# speedup vs baseline: 1.0056x; 1.0056x over previous
"""Trainium2 Bass kernel for EuclideanDistLoss.

reference:
    diff = latent1 - latent2                  # [B, D]
    d = sqrt(sum(diff^2, axis=1))             # [B]
    dev = d - CUTOFF
    penalty = where(dev > 0, dev^2, PRESSURE * dev^2)
    return mean(penalty)

Strategy: data-parallel over the batch dim across 8 NeuronCores. Each core
streams its 32768x256 shard of both inputs through SBUF ([128, k*256] tiles,
k rows per partition), computes per-sample sum-of-squares via DVE subtract ->
ACT Square -> DVE grouped 3D reduce into ssq[128, 256], then one penalty
chain (ACT Sqrt -> DVE cmp/blend -> ACT Square -> DVE mult+reduce) produces
a per-partition partial sum, which the idle GpSimd engine all-reduces
across partitions so a single 4-byte element is DMA'd out per core. The
host sums the 8 scalars in float64 and divides by the global batch (the
"all-reduce" of the scalar).

DMA: the two input streams go through BOTH HWDGE queue sets (a via the SP
engine's qSpDynamicHW, b via the Activation engine's qActDynamicHW), uniform
k=8 tiles (8KB per partition per transfer), single_packet=True on the loads
(bit-exact, measured equal-or-faster: won 6/8 interleaved DMA-only rounds,
~1-3 us). This tracks the concurrent DMA-only bandwidth ceiling (~188-194
us/pass/core, ~355 GB/s/core vs 360 roofline) in every session measured,
whereas routing both streams through one queue set drifts between 190 and
232 us across sessions.

The benchmark (repeat>1) kernels run the identical per-pass instruction
stream, with the penalty chain software-pipelined one pass behind on a
double-buffered ssq: pass r's chain is emitted a few tiles into pass r+1,
and its single-descriptor out-DMA one pass later still, so every
instruction's inputs are long since computed and nothing ever stalls the
in-order HWDGE load queues (measured: chain + out-DMA per pass is free —
the kernel times identically to a DMA-only probe).
"""

import numpy as np

B, D = 262144, 256
N_CORES = 8
P = 128
CUTOFF = 0.1
PRESSURE = 10.0

B_LOCAL = B // N_CORES  # 32768
K_DEFAULT = 8           # rows per partition per tile -> 8KB contiguous / partition
BUFS_DEFAULT = 8
ENGINES_DEFAULT = ("sync", "act")
EMIT_AT = 4             # tile index of pass r+1 where pass r's chain is emitted


def build_nc(b_local=B_LOCAL, k=K_DEFAULT, repeat=1, bufs=BUFS_DEFAULT,
             compute=True, engines=ENGINES_DEFAULT, per_pass_chain=True,
             chain_out_dma=True, single_packet=True):
    """Build + compile the per-core Bass program (SPMD: same program on all cores).

    repeat>1 re-runs the whole pass over the same data (for benchmarking:
    slope of time vs repeat isolates pure on-device time; each pass includes
    the full penalty chain via software pipelining).
    compute=False builds a DMA-only variant (bandwidth ceiling probe).
    """
    import concourse.bacc as bacc
    import concourse.tile as tile
    from concourse import mybir
    from concourse import bass_isa

    f32 = mybir.dt.float32
    Alu = mybir.AluOpType
    Act = mybir.ActivationFunctionType

    if isinstance(k, int):
        tile_rows = P * k
        assert b_local % tile_rows == 0
        schedule = [k] * (b_local // tile_rows)
    else:  # explicit per-tile k schedule
        schedule = list(k)
        assert sum(schedule) * P == b_local
    n = sum(schedule)  # k-units per partition (= penalties per partition)

    nc = bacc.Bacc("TRN2", target_bir_lowering=False, debug=False,
                   num_devices=N_CORES)
    # DMA issuing engines for the a/b input streams. "sync" = SP HWDGE queue
    # set (qSpDynamicHW), "act" = Activation HWDGE queue set (qActDynamicHW),
    # "gpsimd" = Pool SWDGE.
    def _eng(name):
        return {"sync": nc.sync, "act": nc.scalar, "gpsimd": nc.gpsimd}[name]

    a = nc.dram_tensor("latent1", [b_local, D], f32, kind="ExternalInput").ap()
    b = nc.dram_tensor("latent2", [b_local, D], f32, kind="ExternalInput").ap()
    out = nc.dram_tensor("out", [1, 1], f32, kind="ExternalOutput").ap()

    with tile.TileContext(nc) as tc:
        with (
            tc.tile_pool(name="pa", bufs=bufs) as pa,
            tc.tile_pool(name="pb", bufs=bufs) as pb,
            tc.tile_pool(name="keep", bufs=1) as keep,
        ):
            # ssq double-buffered so pass r+1 can stream into one buffer
            # while pass r's penalty chain reads the other.
            ssqs = [keep.tile([P, n], f32, name=f"ssq{i}") for i in range(2)]
            d_ = keep.tile([P, n], f32)
            mask = keep.tile([P, n], f32)  # 1.0 where d < CUTOFF
            fac = keep.tile([P, n], f32)   # 1 + (PRESSURE-1)*mask
            dd = keep.tile([P, n], f32)    # (d - CUTOFF)^2
            pen = keep.tile([P, n], f32)
            # psum/red double-buffered: chain(r) writes psums[r%2], then the
            # idle GpSimd engine all-reduces across partitions into
            # reds[r%2] (every partition holds the core's total). The out-DMA
            # moves ONE 4-byte element, one pass later on the SP queue, when
            # the data is guaranteed ready: a [128,1] out-DMA fragments into
            # 128 tiny descriptors and measured +15us/pass regardless of
            # queue or timing; the single-descriptor form is free.
            psums = [keep.tile([P, 1], f32, name=f"psum{i}") for i in range(2)]
            reds = [keep.tile([P, 1], f32, name=f"red{i}") for i in range(2)]
            neg_cut = keep.tile([P, 1], f32)
            nc.vector.memset(neg_cut, -CUTOFF)

            def penalty_chain(buf):
                # critical path: Sqrt -> Square (both ACT, one table set) ->
                # mult -> reduce; mask/fac run on DVE in parallel with Square.
                ssq = ssqs[buf]
                nc.scalar.activation(out=d_, in_=ssq, func=Act.Sqrt)
                nc.vector.tensor_scalar(mask, d_, CUTOFF, None, Alu.is_lt)
                nc.vector.tensor_scalar(
                    fac, mask, PRESSURE - 1.0, 1.0, Alu.mult, Alu.add
                )
                nc.scalar.activation(
                    out=dd, in_=d_, func=Act.Square, bias=neg_cut[:]
                )
                nc.vector.tensor_tensor(out=pen, in0=dd, in1=fac, op=Alu.mult)
                nc.vector.tensor_reduce(
                    out=psums[buf], in_=pen, axis=mybir.AxisListType.X,
                    op=Alu.add,
                )
                nc.gpsimd.partition_all_reduce(
                    reds[buf][:], psums[buf][:], 128, bass_isa.ReduceOp.add
                )

            if not compute:
                nc.vector.memset(reds[0], 0.0)
                nc.sync.dma_start(out=out, in_=reds[0][0:1, :])
            for r in range(repeat):
                cur = r % 2
                r0 = 0   # row offset within the shard
                c0 = 0   # column offset within ssq
                for i, kt in enumerate(schedule):
                    # partition p holds kt consecutive rows -> contiguous
                    # kt*1KB per partition
                    a_v = a[r0:r0 + P * kt, :].rearrange("(p k) d -> p (k d)", p=P)
                    b_v = b[r0:r0 + P * kt, :].rearrange("(p k) d -> p (k d)", p=P)
                    ta = pa.tile([P, kt * D], f32, tag="ta")
                    tb = pb.tile([P, kt * D], f32, tag="tb")
                    if engines == "split":
                        # each stream split across both queue sets by
                        # partition halves (both queues stay load-balanced
                        # even if one set runs slow)
                        h = P // 2
                        nc.sync.dma_start(out=ta[0:h, :], in_=a_v[0:h, :])
                        nc.scalar.dma_start(out=ta[h:P, :], in_=a_v[h:P, :])
                        nc.scalar.dma_start(out=tb[0:h, :], in_=b_v[0:h, :])
                        nc.sync.dma_start(out=tb[h:P, :], in_=b_v[h:P, :])
                    else:
                        pair = (engines if isinstance(engines, tuple)
                                else engines[i % len(engines)])
                        _eng(pair[0]).dma_start(out=ta, in_=a_v,
                                                single_packet=single_packet)
                        _eng(pair[1]).dma_start(out=tb, in_=b_v,
                                                single_packet=single_packet)
                    r0 += P * kt
                    if not compute:
                        continue
                    nc.vector.tensor_tensor(out=ta, in0=ta, in1=tb, op=Alu.subtract)
                    nc.scalar.activation(out=ta, in_=ta, func=Act.Square)
                    nc.vector.tensor_reduce(
                        out=ssqs[cur][:, c0:c0 + kt],
                        in_=ta.rearrange("p (k d) -> p k d", d=D),
                        axis=mybir.AxisListType.X,
                        op=Alu.add,
                    )
                    c0 += kt
                    if i == EMIT_AT and per_pass_chain and compute:
                        if r >= 2 and chain_out_dma:
                            # chain(r-2)'s total: computed a full pass ago,
                            # single-descriptor DMA, ready on arrival
                            nc.sync.dma_start(out=out, in_=reds[r % 2][0:1, :])
                        if r >= 1:
                            # previous pass's penalty chain: all inputs
                            # ready, hidden under this pass's stream
                            penalty_chain(1 - cur)
            if compute:
                penalty_chain((repeat - 1) % 2)
                nc.sync.dma_start(out=out, in_=reds[(repeat - 1) % 2][0:1, :])

    nc.compile()
    return nc


_NC_CACHE = {}


def _get_nc():
    key = "default"
    if key not in _NC_CACHE:
        _NC_CACHE[key] = build_nc()
    return _NC_CACHE[key]


def run_spmd(latent1, latent2, trace=False, **kwargs):
    """Shard inputs, run on 8 cores, return (scalar_loss, BassKernelResults)."""
    from concourse.bass_utils import run_bass_kernel_spmd

    nc = _get_nc()
    a = np.ascontiguousarray(np.asarray(latent1, dtype=np.float32))
    b = np.ascontiguousarray(np.asarray(latent2, dtype=np.float32))
    assert a.shape == (B, D) and b.shape == (B, D)
    in_maps = [
        {
            "latent1": a[c * B_LOCAL:(c + 1) * B_LOCAL],
            "latent2": b[c * B_LOCAL:(c + 1) * B_LOCAL],
        }
        for c in range(N_CORES)
    ]
    res = run_bass_kernel_spmd(
        nc, in_maps, core_ids=list(range(N_CORES)), trace=trace, **kwargs
    )
    # each core returns its shard's penalty total in out[0, 0]; the host
    # all-reduces the 8 scalars and divides by the global batch
    total = sum(np.asarray(r["out"], dtype=np.float64).sum() for r in res.results)
    return np.asarray(total / B, dtype=np.float32), res


def kernel(latent1, latent2):
    loss, _ = run_spmd(latent1, latent2)
    return loss
